# revision 1
# baseline (speedup 1.0000x reference)
"""Trainium2 Bass kernel for DFConv2d (modulated deformable conv v2).

Sharding: 8 cores = (batch b in 0..3) x (row-half in {0,1}); each core computes
out[b, :, h0:h0+32, :] (2048 positions) from the full image x[b].

v2: software-pipelined per 512-position chunk.  Per chunk:
  A. offset conv (bf16 shifted matmuls) -> om_bf [27, 512-slice]
     -> PE-transpose to position-major (alpha chain on DVE) and wrapped-16
     (index chain on GPSIMD/Pool) maps.  floor() via round(x-0.5) magic
     (exact for bilinear: off-by-one floor gives weight-1.0 on the other
     corner).  Indices written int16 into partitions 0:15 only (SWDGE reads
     only the first 16 partitions of the idx AP).
  B. per (tap): dma_gather of bf16 channel-pair vectors from the zero-padded
     channels-last HBM image -> per-partition-scalar bilinear weighting (DVE)
     -> PE transpose to channel-major -> bf16 matmuls accumulating over
     taps/channel-groups in PSUM -> bf16 out (host casts to f32).
Chunk n+1's stage A is emitted just after chunk n's first tap so every
engine queue stays fed while the 36 gathers stream back-to-back on DMA.
"""
import os
import sys

sys.path.insert(0, "/opt/trn_rl_repo")

import numpy as np
import ml_dtypes

import concourse.bass as bass
import concourse.tile as tile
from concourse import bacc, mybir
from concourse.bass_utils import run_bass_kernel_spmd
from concourse.masks import make_identity

F32 = mybir.dt.float32
BF16 = mybir.dt.bfloat16
I16 = mybir.dt.int16
AF = mybir.ActivationFunctionType
ALU = mybir.AluOpType

B, C, H, W = 4, 256, 64, 64
COUT = 256
HP, WP = 68, 69      # padded channels-last image dims
NROW = HP * WP       # 4692
M0 = 12582912.0      # 1.5 * 2**23 (round-to-nearest magic)

_BUILD_CACHE = {}


def _rap(base, off, pairs, part=None):
    """Raw AP on a tile's tensor: partition pair from base, custom free pairs."""
    p0 = list(base.ap[0])
    if part is not None:
        p0 = [p0[0], part]
    return bass.AP(tensor=base.tensor, offset=base.offset + off,
                   ap=[p0] + [list(p) for p in pairs])


def _emit(tc):
    nc = tc.nc
    V = nc.vector
    G = nc.gpsimd

    # ---- I/O ----
    xslab = nc.declare_dram_parameter("xslab", [128, 2, 34, 66], BF16, isOutput=False)
    xcl = nc.declare_dram_parameter("xcl", [NROW, 256], BF16, isOutput=False)
    woff = nc.declare_dram_parameter("woff", [128, 2, 9, 27], BF16, isOutput=False)
    wconv = nc.declare_dram_parameter("wconv", [128, 2, 9, 2, 128], BF16, isOutput=False)
    boff = nc.declare_dram_parameter("boff", [27, 1], F32, isOutput=False)
    # wrapped-replicated base tables, pre-shifted by -M0: [p128, n4, sub4, ph8, t9]
    bymt = nc.declare_dram_parameter("bymt", [128, 4, 4, 8, 9], F32, isOutput=False)
    bxmt = nc.declare_dram_parameter("bxmt", [128, 4, 4, 8, 9], F32, isOutput=False)
    selwt = nc.declare_dram_parameter("selwt", [16, 128], BF16, isOutput=False)
    out = nc.declare_dram_parameter("out", [128, 2, 2048], BF16, isOutput=True)

    singles = tc.alloc_tile_pool(name="singles", bufs=1)

    # ---- constants / inputs to SBUF ----
    # load order: om-conv dependencies first so chunk 0 starts ASAP
    wo = singles.tile([128, 2, 9, 27], BF16)
    nc.sync.dma_start(out=wo, in_=woff[:, :, :, :])
    bo = singles.tile([27, 1], F32)
    nc.sync.dma_start(out=bo, in_=boff[:, :])
    xs = singles.tile([128, 2, 34, 66], BF16)
    nc.sync.dma_start(out=xs[:, :, 0:11, :], in_=xslab[:, :, 0:11, :])
    bym = singles.tile([128, 4, 4, 8, 9], F32)
    nc.sync.dma_start(out=bym, in_=bymt[:, :, :, :, :])
    bxm = singles.tile([128, 4, 4, 8, 9], F32)
    nc.sync.dma_start(out=bxm, in_=bxmt[:, :, :, :, :])
    nc.sync.dma_start(out=xs[:, :, 11:34, :], in_=xslab[:, :, 11:34, :])
    wc = singles.tile([128, 2, 9, 2, 128], BF16)
    nc.sync.dma_start(out=wc, in_=wconv[:, :, :, :, :])

    identb = singles.tile([128, 128], BF16)
    make_identity(nc, identb)
    selw = singles.tile([16, 128], BF16)
    nc.sync.dma_start(out=selw, in_=selwt[:, :])

    # ---- persistent stage-A tensors ----
    om_bf = singles.tile([27, 2048], BF16)            # bias-added offset conv out
    omT = singles.tile([128, 16, 27], F32)            # position-major om^T
    al = singles.tile([128, 4, 16, 9], F32)           # bilinear corner weights
    omTr = singles.tile([128, 4, 4, 8, 27], F32)      # wrapped-replicated om^T
    idxi = singles.tile([128, 9, 4, 2, 4, 8], I16)    # gather indices
    outsb = singles.tile([128, 2, 2048], BF16)


    mp_pool = tc.alloc_tile_pool(name="mp", bufs=2)
    chain_pool = tc.alloc_tile_pool(name="chain", bufs=1)

    pom_pool = tc.alloc_tile_pool(name="pom", bufs=1, space="PSUM")
    pt_pool = tc.alloc_tile_pool(name="pt", bufs=1, space="PSUM")
    pti_pool = tc.alloc_tile_pool(name="pti", bufs=1, space="PSUM")
    pg_pool = tc.alloc_tile_pool(name="pg", bufs=3, space="PSUM")
    po_pool = tc.alloc_tile_pool(name="po", bufs=2, space="PSUM")

    gpool = tc.alloc_tile_pool(name="gout", bufs=3)
    wpool = tc.alloc_tile_pool(name="wtiles", bufs=10)
    gsbp = tc.alloc_tile_pool(name="gsbp", bufs=4)

    # warm up the PE p-state tracker (~3.5us of continuous junk matmuls)
    # so chunk 0's offset conv runs at full clock instead of 788ns/matmul
    pw = pg_pool.tile([128, 512], F32, tag="pg", name="warm")
    for _ in range(30):
        nc.tensor.matmul(pw[:, 0:128], lhsT=identb[:, :], rhs=identb[:, :],
                         start=True, stop=True)

    def stage_a(n):
        """Offset conv + alpha/index maps for 512-position chunk n."""
        # offset conv: accumulate 18 shifted bf16 matmuls into PSUM
        pom = pom_pool.tile([27, 512], F32, tag="pom")
        i = 0
        for cg in range(2):
            for k in range(9):
                kh, kw = k // 3, k % 3
                roff = cg * (34 * 66) + (kh + n * 8) * 66 + kw
                rhs = _rap(xs[:], roff, [[66, 8], [1, 64]])
                nc.tensor.matmul(
                    pom[:, :], lhsT=wo[:, cg, k, :], rhs=rhs,
                    start=(i == 0), stop=(i == 17))
                i += 1
        nc.scalar.activation(
            out=om_bf[0:27, n * 512:(n + 1) * 512], in_=pom[0:27, :],
            func=AF.Identity, bias=bo[0:27, :], scale=1.0)

        # position-major om^T (4 pos-groups of 128)
        pomT = pt_pool.tile([128, 4, 28], BF16, tag="pt")
        for i4 in range(4):
            ch = n * 4 + i4
            nc.tensor.matmul(
                pomT[:, i4, 0:27],
                lhsT=om_bf[0:27, ch * 128:(ch + 1) * 128],
                rhs=identb[0:27, 0:27],
                is_transpose=True, start=True, stop=True)
        nc.scalar.copy(
            _rap(omT[:], n * 4 * 27, [[27, 4], [1, 27]]),
            _rap(pomT[:], 0, [[28, 4], [1, 27]]))

        # alpha chain (views [128, 4ch, 9t])
        def omt_view(j0, step, cnt):
            return _rap(omT[:], n * 108 + j0, [[27, 4], [step, cnt]])

        dyv = omt_view(0, 2, 9)
        dxv = omt_view(1, 2, 9)
        mp = mp_pool.tile([128, 6, 4, 9], F32, tag="mp")
        msig = mp_pool.tile([128, 4, 9], F32, tag="msig")
        nc.scalar.activation(out=msig[:], in_=omt_view(18, 1, 9), func=AF.Sigmoid)
        aln = al[:, :, n * 4:(n + 1) * 4, :]
        V.tensor_scalar(mp[:, 0], dyv, 0.5, M0, ALU.subtract, ALU.add)
        V.tensor_scalar(mp[:, 0], mp[:, 0], M0, None, ALU.subtract)
        V.tensor_tensor(out=mp[:, 1], in0=dyv, in1=mp[:, 0], op=ALU.subtract)  # wy
        V.tensor_scalar(mp[:, 2], dxv, 0.5, M0, ALU.subtract, ALU.add)
        V.tensor_scalar(mp[:, 2], mp[:, 2], M0, None, ALU.subtract)
        V.tensor_tensor(out=mp[:, 2], in0=dxv, in1=mp[:, 2], op=ALU.subtract)  # wx
        V.tensor_tensor(out=mp[:, 4], in0=msig[:], in1=mp[:, 1], op=ALU.mult)   # m*wy
        V.tensor_tensor(out=mp[:, 3], in0=msig[:], in1=mp[:, 4], op=ALU.subtract)  # m*(1-wy)
        V.tensor_tensor(out=aln[:, 1], in0=mp[:, 3], in1=mp[:, 2], op=ALU.mult)
        V.tensor_tensor(out=aln[:, 0], in0=mp[:, 3], in1=aln[:, 1], op=ALU.subtract)
        V.tensor_tensor(out=aln[:, 3], in0=mp[:, 4], in1=mp[:, 2], op=ALU.mult)
        V.tensor_tensor(out=aln[:, 2], in0=mp[:, 4], in1=aln[:, 3], op=ALU.subtract)

        # wrapped om^T on 16 partitions, then replicate to all 8 partition
        # groups with a selection-matrix matmul (selw[k,p] = [p%16==k]) so
        # the chain runs on 128 partitions and idxi needs no broadcast
        pomTi = pti_pool.tile([16, 32, 28], BF16, tag="pti")
        for i4 in range(4):
            ch = n * 4 + i4
            for ph in range(8):
                s = i4 * 8 + ph
                nc.tensor.matmul(
                    pomTi[:, s, 0:27],
                    lhsT=om_bf[0:27, ch * 128 + ph * 16: ch * 128 + ph * 16 + 16],
                    rhs=identb[0:27, 0:27],
                    is_transpose=True, start=True, stop=True)
        oti = mp_pool.tile([16, 864], BF16, tag="oti")
        nc.scalar.copy(
            _rap(oti[:], 0, [[27, 32], [1, 27]]),
            _rap(pomTi[:], 0, [[28, 32], [1, 27]]))
        for hf in range(2):
            prepl = pg_pool.tile([128, 512], F32, tag="pg", name="repl")
            nc.tensor.matmul(prepl[:, 0:432], lhsT=selw[:, :],
                             rhs=oti[:, hf * 432:(hf + 1) * 432],
                             start=True, stop=True)
            nc.scalar.copy(
                _rap(omTr[:], n * 864 + hf * 432, [[1, 432]]),
                prepl[:, 0:432])

        # index chain on GPSIMD (views [128, 4sub, 8ph, 9t], free 288)
        def omti_view(j0, step, cnt):
            return _rap(omTr[:], n * 864 + j0, [[216, 4], [27, 8], [step, cnt]])

        dyv2 = omti_view(0, 2, 9)
        dxv2 = omti_view(1, 2, 9)
        ry = chain_pool.tile([128, 4, 8, 9], F32, tag="ry")
        uf = chain_pool.tile([128, 4, 8, 9], F32, tag="uf")
        y0 = chain_pool.tile([128, 4, 8, 9], F32, tag="y0")
        y1m = chain_pool.tile([128, 4, 8, 9], F32, tag="y1m")
        x0 = chain_pool.tile([128, 4, 8, 9], F32, tag="x0")
        G.tensor_scalar(ry[:], dyv2, 0.5, M0, ALU.subtract, ALU.add)
        G.tensor_tensor(out=uf[:], in0=ry[:], in1=bym[:, n], op=ALU.add)
        G.tensor_scalar(y0[:], uf[:], 0.0, 67.0, ALU.max, ALU.min)
        G.tensor_scalar(y1m[:], uf[:], -1.0, 66.0, ALU.max, ALU.min)  # y1 - 1
        G.tensor_scalar(ry[:], dxv2, 0.5, M0, ALU.subtract, ALU.add)
        G.tensor_tensor(out=uf[:], in0=ry[:], in1=bxm[:, n], op=ALU.add)
        G.tensor_scalar(x0[:], uf[:], 0.0, 67.0, ALU.max, ALU.min)
        G.tensor_scalar(y0[:], y0[:], 69.0, None, ALU.mult)
        G.tensor_tensor(out=y0[:], in0=y0[:], in1=x0[:], op=ALU.add)    # t1
        G.tensor_scalar(y1m[:], y1m[:], 69.0, 69.0, ALU.mult, ALU.add)
        G.tensor_tensor(out=y1m[:], in0=y1m[:], in1=x0[:], op=ALU.add)  # t2
        # int16 convert into idxi (all 128 partitions); src iter (sub, ph, t)
        for rc, src in ((0, y0), (1, y1m)):
            sv = _rap(src[:], 0, [[72, 4], [9, 8], [1, 9]])
            dst = _rap(idxi[:], n * 64 + rc * 32, [[8, 4], [1, 8], [256, 9]])
            G.tensor_copy(out=dst, in_=sv)

    # =================== pipelined main loop ===================
    xcl_base = xcl.ap()
    xcl_rows = bass.AP(tensor=xcl_base.tensor, offset=0, ap=[[256, NROW - 1], [1, 512]])

    stage_a(0)
    for n in range(4):
        pouts = [po_pool.tile([128, 512], F32, tag="pout", name=f"pout{_og}")
                 for _og in range(2)]
        for t in range(9):
            go = gpool.tile([128, 8, 512], BF16, tag="go")
            nc.gpsimd.dma_gather(
                out_ap=go[:],
                in_ap=xcl_rows,
                idxs_ap=idxi[:, t, n, :, :, :],
                num_idxs=1024,
                num_idxs_reg=1024,
                elem_size=512,
                elem_step=256,
            )
            pgs = [pg_pool.tile([128, 512], F32, tag="pg", name=f"pg{_cg}")
                   for _cg in range(2)]
            for sub in range(4):
                ch = n * 4 + sub
                tb = wpool.tile([128, 4, 256], BF16, tag="tb", name="tb")
                s0 = wpool.tile([128, 256], BF16, tag="s0", name="s0")
                V.tensor_scalar_mul(tb[:, 0], go[:, 0 + sub, 0:256], al[:, 0, ch, t:t + 1])
                V.tensor_scalar_mul(tb[:, 1], go[:, 0 + sub, 256:512], al[:, 1, ch, t:t + 1])
                V.tensor_scalar_mul(tb[:, 2], go[:, 4 + sub, 0:256], al[:, 2, ch, t:t + 1])
                V.tensor_scalar_mul(tb[:, 3], go[:, 4 + sub, 256:512], al[:, 3, ch, t:t + 1])
                V.tensor_tensor(out=s0[:], in0=tb[:, 0], in1=tb[:, 1], op=ALU.add)
                # transpose-by-matmul against identity: fp32 PSUM accumulation
                for cg in range(2):
                    for pi, piece in enumerate((s0[:], tb[:, 2], tb[:, 3])):
                        nc.tensor.matmul(
                            pgs[cg][:, sub * 128:(sub + 1) * 128],
                            lhsT=piece[:, cg * 128:(cg + 1) * 128],
                            rhs=identb[:, :],
                            start=(pi == 0), stop=(pi == 2))
            for cg in range(2):
                gsb = gsbp.tile([128, 512], BF16, tag="gsb")
                nc.scalar.copy(gsb[:], pgs[cg][:])
                for og in range(2):
                    nc.tensor.matmul(
                        pouts[og][:, :],
                        lhsT=wc[:, cg, t, og, :],
                        rhs=gsb[:],
                        start=(t == 0 and cg == 0),
                        stop=(t == 8 and cg == 1),
                    )
            if t == 0 and n < 3:
                stage_a(n + 1)
        for og in range(2):
            nc.scalar.copy(outsb[:, og, n * 512:(n + 1) * 512], pouts[og][:])
        nc.sync.dma_start(out=out[:, :, n * 512:(n + 1) * 512],
                          in_=outsb[:, :, n * 512:(n + 1) * 512])

    for p in (gsbp, wpool, gpool, po_pool, pg_pool, pti_pool, pt_pool,
              pom_pool, chain_pool, mp_pool, singles):
        p.release()


def _build():
    if "nc" in _BUILD_CACHE:
        return _BUILD_CACHE["nc"]
    nc = bacc.Bacc("TRN2", target_bir_lowering=False, debug=False, num_devices=8)
    with tile.TileContext(nc) as tc:
        _emit(tc)
    nc.compile()
    _BUILD_CACHE["nc"] = nc
    return nc


def _host_prep(x, w_off, b_off, w_conv):
    x = np.asarray(x, np.float32)
    w_off = np.asarray(w_off, np.float32)
    b_off = np.asarray(b_off, np.float32)
    w_conv = np.asarray(w_conv, np.float32)

    wof = w_off.reshape(27, 2, 128, 9)                       # [j, cg, cp, k]
    woff_sb = np.ascontiguousarray(
        np.transpose(wof, (2, 1, 3, 0))).astype(ml_dtypes.bfloat16)
    wcv = w_conv.reshape(2, 128, 2, 128, 9)                  # [og, op, cg, cp, k]
    wconv_sb = np.ascontiguousarray(
        np.transpose(wcv, (3, 2, 4, 0, 1))).astype(ml_dtypes.bfloat16)
    boff_sb = np.ascontiguousarray(b_off.reshape(27, 1))

    # wrapped-replicated base tables [p128, n4, sub4, ph8, t9], pre-shifted by -M0
    pl = np.arange(128)[:, None, None, None, None] % 16
    nv = np.arange(4)[None, :, None, None, None]
    sv = np.arange(4)[None, None, :, None, None]
    phv = np.arange(8)[None, None, None, :, None]
    tv = np.arange(9)[None, None, None, None, :]
    pos = (nv * 4 + sv) * 128 + phv * 16 + pl                # [128,4,4,8,1]
    kh = tv // 3
    kw = tv % 3
    selw_bf = np.ascontiguousarray(
        (np.arange(128)[None, :] % 16 == np.arange(16)[:, None])
        .astype(ml_dtypes.bfloat16))

    in_maps = []
    for b in range(B):
        xcl = np.zeros((HP, WP, 256), np.float32)
        xcl[2:66, 2:66, :] = np.transpose(x[b], (1, 2, 0))
        xcl_bf = np.ascontiguousarray(xcl.astype(ml_dtypes.bfloat16).reshape(NROW, 256))
        for half in range(2):
            h0 = half * 32
            hh = h0 + pos // 64
            ww = pos % 64
            bym = np.ascontiguousarray(
                np.broadcast_to(hh + kh + 1, (128, 4, 4, 8, 9)).astype(np.float64)
                - M0).astype(np.float32)
            bxm = np.ascontiguousarray(
                np.broadcast_to(ww + kw + 1, (128, 4, 4, 8, 9)).astype(np.float64)
                - M0).astype(np.float32)
            xslab = np.zeros((256, 34, 66), np.float32)
            r_lo = h0 - 1
            src_lo, src_hi = max(r_lo, 0), min(h0 + 33, H)
            xslab[:, src_lo - r_lo: src_hi - r_lo, 1:65] = x[b][:, src_lo:src_hi, :]
            xslab_sb = np.ascontiguousarray(
                np.transpose(xslab.reshape(2, 128, 34, 66), (1, 0, 2, 3))
            ).astype(ml_dtypes.bfloat16)
            in_maps.append({
                "xslab": xslab_sb,
                "xcl": xcl_bf,
                "woff": woff_sb,
                "wconv": wconv_sb,
                "boff": boff_sb,
                "bymt": bym,
                "bxmt": bxm,
                "selwt": selw_bf,
            })
    return in_maps


def kernel(**inputs):
    x = np.asarray(inputs["x"])
    in_maps = _host_prep(x, inputs["w_off"], inputs["b_off"], inputs["w_conv"])
    nc = _build()
    res = run_bass_kernel_spmd(nc, in_maps, core_ids=list(range(8)))
    out = np.zeros((B, COUT, H, W), np.float32)
    for core in range(8):
        b, half = core // 2, core % 2
        r = np.asarray(res.results[core]["out"]).astype(np.float32)
        o = np.transpose(r, (1, 0, 2)).reshape(COUT, 32, 64)
        out[b, :, half * 32:(half + 1) * 32, :] = o
    return out



# revision 8
# speedup vs baseline: 1.0649x; 1.0649x over previous
"""Trainium2 Bass kernel for DFConv2d (modulated deformable conv v2).

Sharding: 8 cores = (batch b in 0..3) x (row-half in {0,1}); each core computes
out[b, :, h0:h0+32, :] (2048 positions) from the full image x[b].

v3: row-pair-interleaved gather image.  The padded channels-last image is
stored in DRAM as E[r] = concat(row r, row r+WP) (512 ch per entry), so ONE
gather index fetches all four bilinear corners (2 KB: c00,c10,c01,c11).
Per half-chunk (256 positions) a single dma_gather call covers all 9 taps
(2304 idxs) - 8 gather calls total, back-to-back on the DMA engines (the
model's shared 360 GB/s resource; gathers are the 105 us floor).  Index
chain needs only r = y0*WP + x0.  floor() via round(x-0.5) magic (exact
for bilinear: off-by-one floor gives weight-1.0 on the other corner).
Corner weighting: per-partition-scalar DVE mults (4x mode) in
position-major; corner sum rides on PE transpose-matmul PSUM accumulation
(3-piece for s2=0 with the pre-add on DVE, 4-piece for s2=1, balancing
DVE vs PE).  Compute is half-chunk granular so go bufs=2 keeps the DMA
saturated.
"""
import os
import sys

sys.path.insert(0, "/opt/trn_rl_repo")

import numpy as np
import ml_dtypes

import concourse.bass as bass
import concourse.tile as tile
from concourse import bacc, mybir
from concourse.bass_utils import run_bass_kernel_spmd
from concourse.masks import make_identity

F32 = mybir.dt.float32
BF16 = mybir.dt.bfloat16
I16 = mybir.dt.int16
AF = mybir.ActivationFunctionType
ALU = mybir.AluOpType

B, C, H, W = 4, 256, 64, 64
COUT = 256
HP, WP = 68, 69      # padded channels-last image dims
NROW = HP * WP       # 4692
M0 = 12582912.0      # 1.5 * 2**23 (round-to-nearest magic)

_BUILD_CACHE = {}


def _rap(base, off, pairs, part=None):
    """Raw AP on a tile's tensor: partition pair from base, custom free pairs."""
    p0 = list(base.ap[0])
    if part is not None:
        p0 = [p0[0], part]
    return bass.AP(tensor=base.tensor, offset=base.offset + off,
                   ap=[p0] + [list(p) for p in pairs])


def _emit(tc):
    nc = tc.nc
    V = nc.vector
    G = nc.gpsimd

    # ---- I/O ----
    # xslab stored flat-contiguous per partition for full-bandwidth load
    xslab = nc.declare_dram_parameter("xslab", [128, 4488], BF16, isOutput=False)
    # row-pair-interleaved gather image: E[r] = concat(xcl[r], xcl[r+WP])
    xrp = nc.declare_dram_parameter("xrp", [NROW, 512], BF16, isOutput=False)
    woff = nc.declare_dram_parameter("woff", [128, 2, 9, 27], BF16, isOutput=False)
    wconv = nc.declare_dram_parameter("wconv", [128, 2, 9, 2, 128], BF16, isOutput=False)
    boff = nc.declare_dram_parameter("boff", [27, 1], F32, isOutput=False)
    # wrapped-replicated base tables, pre-shifted by -M0: [p128, n4, sub4, ph8, t9]
    bymt = nc.declare_dram_parameter("bymt", [128, 4, 4, 8, 9], F32, isOutput=False)
    bxmt = nc.declare_dram_parameter("bxmt", [128, 4, 4, 8, 9], F32, isOutput=False)
    selwt = nc.declare_dram_parameter("selwt", [16, 128], BF16, isOutput=False)
    out = nc.declare_dram_parameter("out", [128, 2, 2048], BF16, isOutput=True)

    singles = tc.alloc_tile_pool(name="singles", bufs=1)

    # ---- constants / inputs to SBUF ----
    # load order: om-conv dependencies first so chunk 0 starts ASAP.
    # om conv for chunk 0 needs xslab rows 0..10 of both channel groups:
    # flat offsets [0, 726) and [2244, 2970).
    wo = singles.tile([128, 2, 9, 27], BF16)
    nc.sync.dma_start(out=wo, in_=woff[:, :, :, :])
    bo = singles.tile([27, 1], F32)
    nc.sync.dma_start(out=bo, in_=boff[:, :])
    xs = singles.tile([128, 2, 34, 66], BF16)
    nc.sync.dma_start(out=_rap(xs[:], 0, [[1, 726]]),
                      in_=_rap(xslab.ap(), 0, [[1, 726]]))
    nc.sync.dma_start(out=_rap(xs[:], 2244, [[1, 726]]),
                      in_=_rap(xslab.ap(), 2244, [[1, 726]]))
    bym = singles.tile([128, 4, 4, 8, 9], F32)
    nc.sync.dma_start(out=bym, in_=bymt[:, :, :, :, :])
    bxm = singles.tile([128, 4, 4, 8, 9], F32)
    nc.sync.dma_start(out=bxm, in_=bxmt[:, :, :, :, :])
    # rest of xslab: [726, 2244) and [2970, 4488)
    nc.sync.dma_start(out=_rap(xs[:], 726, [[1, 1518]]),
                      in_=_rap(xslab.ap(), 726, [[1, 1518]]))
    nc.sync.dma_start(out=_rap(xs[:], 2970, [[1, 1518]]),
                      in_=_rap(xslab.ap(), 2970, [[1, 1518]]))
    wc = singles.tile([128, 2, 9, 2, 128], BF16)
    nc.sync.dma_start(out=wc, in_=wconv[:, :, :, :, :])

    identb = singles.tile([128, 128], BF16)
    make_identity(nc, identb)
    selw = singles.tile([16, 128], BF16)
    nc.sync.dma_start(out=selw, in_=selwt[:, :])

    # ---- persistent stage-A tensors ----
    om_bf = singles.tile([27, 2048], BF16)            # bias-added offset conv out
    omT = singles.tile([128, 16, 27], F32)            # position-major om^T
    al = singles.tile([128, 4, 16, 9], F32)           # bilinear corner weights
    omTr = singles.tile([128, 4, 4, 8, 27], F32)      # wrapped-replicated om^T
    # gather indices: [p128(16 used), n4, h2, t9, s2 2, pb8]; free slot within
    # a half-chunk's 144-slot list is t*16 + s2*8 + pb
    idxi = singles.tile([128, 4, 2, 9, 2, 8], I16)
    outsb = singles.tile([128, 2, 2048], BF16)

    mp_pool = tc.alloc_tile_pool(name="mp", bufs=2)
    chain_pool = tc.alloc_tile_pool(name="chain", bufs=1)

    pom_pool = tc.alloc_tile_pool(name="pom", bufs=1, space="PSUM")
    pt_pool = tc.alloc_tile_pool(name="pt", bufs=1, space="PSUM")
    pti_pool = tc.alloc_tile_pool(name="pti", bufs=1, space="PSUM")
    pg_pool = tc.alloc_tile_pool(name="pg", bufs=3, space="PSUM")
    po_pool = tc.alloc_tile_pool(name="po", bufs=2, space="PSUM")

    gpool = tc.alloc_tile_pool(name="gout", bufs=2)
    wpool = tc.alloc_tile_pool(name="wtiles", bufs=10)
    gsbp = tc.alloc_tile_pool(name="gsbp", bufs=4)

    # warm up the PE p-state tracker (~3.5us of continuous junk matmuls)
    # so chunk 0's offset conv runs at full clock instead of 788ns/matmul
    pw = pg_pool.tile([128, 512], F32, tag="pg", name="warm")
    for _ in range(30):
        nc.tensor.matmul(pw[:, 0:128], lhsT=identb[:, :], rhs=identb[:, :],
                         start=True, stop=True)

    def stage_a(n):
        """Offset conv + alpha/index maps for 512-position chunk n."""
        # offset conv: accumulate 18 shifted bf16 matmuls into PSUM
        pom = pom_pool.tile([27, 512], F32, tag="pom")
        i = 0
        for cg in range(2):
            for k in range(9):
                kh, kw = k // 3, k % 3
                roff = cg * (34 * 66) + (kh + n * 8) * 66 + kw
                rhs = _rap(xs[:], roff, [[66, 8], [1, 64]])
                nc.tensor.matmul(
                    pom[:, :], lhsT=wo[:, cg, k, :], rhs=rhs,
                    start=(i == 0), stop=(i == 17))
                i += 1
        nc.scalar.activation(
            out=om_bf[0:27, n * 512:(n + 1) * 512], in_=pom[0:27, :],
            func=AF.Identity, bias=bo[0:27, :], scale=1.0)

        # position-major om^T (4 pos-groups of 128)
        pomT = pt_pool.tile([128, 4, 28], BF16, tag="pt")
        for i4 in range(4):
            ch = n * 4 + i4
            nc.tensor.matmul(
                pomT[:, i4, 0:27],
                lhsT=om_bf[0:27, ch * 128:(ch + 1) * 128],
                rhs=identb[0:27, 0:27],
                is_transpose=True, start=True, stop=True)
        nc.scalar.copy(
            _rap(omT[:], n * 4 * 27, [[27, 4], [1, 27]]),
            _rap(pomT[:], 0, [[28, 4], [1, 27]]))

        # alpha chain (views [128, 4ch, 9t])
        def omt_view(j0, step, cnt):
            return _rap(omT[:], n * 108 + j0, [[27, 4], [step, cnt]])

        dyv = omt_view(0, 2, 9)
        dxv = omt_view(1, 2, 9)
        mp = mp_pool.tile([128, 6, 4, 9], F32, tag="mp")
        msig = mp_pool.tile([128, 4, 9], F32, tag="msig")
        nc.scalar.activation(out=msig[:], in_=omt_view(18, 1, 9), func=AF.Sigmoid)
        aln = al[:, :, n * 4:(n + 1) * 4, :]
        V.tensor_scalar(mp[:, 0], dyv, 0.5, M0, ALU.subtract, ALU.add)
        V.tensor_scalar(mp[:, 0], mp[:, 0], M0, None, ALU.subtract)
        V.tensor_tensor(out=mp[:, 1], in0=dyv, in1=mp[:, 0], op=ALU.subtract)  # wy
        V.tensor_scalar(mp[:, 2], dxv, 0.5, M0, ALU.subtract, ALU.add)
        V.tensor_scalar(mp[:, 2], mp[:, 2], M0, None, ALU.subtract)
        V.tensor_tensor(out=mp[:, 2], in0=dxv, in1=mp[:, 2], op=ALU.subtract)  # wx
        V.tensor_tensor(out=mp[:, 4], in0=msig[:], in1=mp[:, 1], op=ALU.mult)   # m*wy
        V.tensor_tensor(out=mp[:, 3], in0=msig[:], in1=mp[:, 4], op=ALU.subtract)  # m*(1-wy)
        V.tensor_tensor(out=aln[:, 1], in0=mp[:, 3], in1=mp[:, 2], op=ALU.mult)
        V.tensor_tensor(out=aln[:, 0], in0=mp[:, 3], in1=aln[:, 1], op=ALU.subtract)
        V.tensor_tensor(out=aln[:, 3], in0=mp[:, 4], in1=mp[:, 2], op=ALU.mult)
        V.tensor_tensor(out=aln[:, 2], in0=mp[:, 4], in1=aln[:, 3], op=ALU.subtract)

        # wrapped om^T on 16 partitions, then replicate to all 8 partition
        # groups with a selection-matrix matmul (selw[k,p] = [p%16==k]) so
        # the chain runs on 128 partitions and idxi needs no broadcast
        pomTi = pti_pool.tile([16, 32, 28], BF16, tag="pti")
        for i4 in range(4):
            ch = n * 4 + i4
            for ph in range(8):
                s = i4 * 8 + ph
                nc.tensor.matmul(
                    pomTi[:, s, 0:27],
                    lhsT=om_bf[0:27, ch * 128 + ph * 16: ch * 128 + ph * 16 + 16],
                    rhs=identb[0:27, 0:27],
                    is_transpose=True, start=True, stop=True)
        oti = mp_pool.tile([16, 864], BF16, tag="oti")
        nc.scalar.copy(
            _rap(oti[:], 0, [[27, 32], [1, 27]]),
            _rap(pomTi[:], 0, [[28, 32], [1, 27]]))
        for hf in range(2):
            prepl = pg_pool.tile([128, 512], F32, tag="pg", name="repl")
            nc.tensor.matmul(prepl[:, 0:432], lhsT=selw[:, :],
                             rhs=oti[:, hf * 432:(hf + 1) * 432],
                             start=True, stop=True)
            nc.scalar.copy(
                _rap(omTr[:], n * 864 + hf * 432, [[1, 432]]),
                prepl[:, 0:432])

        # index chain on GPSIMD (views [128, 4sub, 8ph, 9t], free 288).
        # single idx per (pos, tap): r = clamp(y0p, 0, 66)*69 + clamp(x0p, 0, 67)
        # bym/bxm hold (base+pad - M0) so the round magic folds in.
        def omti_view(j0, step, cnt):
            return _rap(omTr[:], n * 864 + j0, [[216, 4], [27, 8], [step, cnt]])

        dyv2 = omti_view(0, 2, 9)
        dxv2 = omti_view(1, 2, 9)
        ry = chain_pool.tile([128, 4, 8, 9], F32, tag="ry")
        uf = chain_pool.tile([128, 4, 8, 9], F32, tag="uf")
        y0 = chain_pool.tile([128, 4, 8, 9], F32, tag="y0")
        x0 = chain_pool.tile([128, 4, 8, 9], F32, tag="x0")
        G.tensor_scalar(ry[:], dyv2, 0.5, M0, ALU.subtract, ALU.add)
        G.tensor_tensor(out=uf[:], in0=ry[:], in1=bym[:, n], op=ALU.add)
        G.tensor_scalar(y0[:], uf[:], 0.0, 66.0, ALU.max, ALU.min)
        G.tensor_scalar(ry[:], dxv2, 0.5, M0, ALU.subtract, ALU.add)
        G.tensor_tensor(out=uf[:], in0=ry[:], in1=bxm[:, n], op=ALU.add)
        G.tensor_scalar(x0[:], uf[:], 0.0, 67.0, ALU.max, ALU.min)
        G.tensor_scalar(y0[:], y0[:], 69.0, None, ALU.mult)
        G.tensor_tensor(out=y0[:], in0=y0[:], in1=x0[:], op=ALU.add)    # r
        # int16 convert into idxi; src iter (s2, ph->pb, t); sub = h*2+s2.
        for h in range(2):
            sv = _rap(y0[:], h * 144, [[72, 2], [9, 8], [1, 9]])
            dst = _rap(idxi[:], n * 288 + h * 144, [[8, 2], [1, 8], [16, 9]])
            G.tensor_copy(out=dst, in_=sv)

    # =================== pipelined main loop ===================
    xrp_base = xrp.ap()
    xrp_rows = bass.AP(tensor=xrp_base.tensor, offset=0,
                       ap=[[512, NROW - 1], [1, 1024]])

    # corner order within a gathered 2KB block: c00@0, c10@256, c01@512, c11@768
    # al corner order: 0:(1-wy)(1-wx)m -> c00, 1:(1-wy)wx*m -> c01,
    #                  2:wy(1-wx)m -> c10, 3:wy*wx*m -> c11
    CPOS = (0, 512, 256, 768)

    def emit_gather(n, h):
        go = gpool.tile([128, 18, 1024], BF16, tag="go")
        nc.gpsimd.dma_gather(
            out_ap=go[:],
            in_ap=xrp_rows,
            idxs_ap=idxi[:, n, h, :, :, :],
            num_idxs=2304,
            num_idxs_reg=2304,
            elem_size=1024,
            elem_step=512,
        )
        return go

    def emit_compute(n, h, go):
        """Weight + transpose + conv for half-chunk (n, h): 256 positions."""
        pouts = po_pool.tile([128, 2, 256], F32, tag="pout", name="pout")
        for t in range(9):
            pgs = pg_pool.tile([128, 2, 256], F32, tag="pg", name="pgs")
            for s2 in range(2):
                sub = h * 2 + s2
                ch = n * 4 + sub
                slot = t * 2 + s2
                tb = wpool.tile([128, 4, 256], BF16, tag="tb", name="tb")
                s0 = wpool.tile([128, 256], BF16, tag="s0", name="s0")
                for c4 in range(4):
                    V.tensor_scalar_mul(
                        tb[:, c4], go[:, slot, CPOS[c4]:CPOS[c4] + 256],
                        al[:, c4, ch, t:t + 1])
                # balance DVE vs PE: s2=0 pre-adds on DVE (3-piece transpose),
                # s2=1 uses 4-piece PSUM accumulation on PE
                if s2 == 0:
                    V.tensor_tensor(out=s0[:], in0=tb[:, 0], in1=tb[:, 1],
                                    op=ALU.add)
                    pieces = (s0[:], tb[:, 2], tb[:, 3])
                else:
                    pieces = (tb[:, 0], tb[:, 1], tb[:, 2], tb[:, 3])
                # transpose-by-matmul against identity: fp32 PSUM accumulation
                for cg in range(2):
                    for pi, piece in enumerate(pieces):
                        nc.tensor.matmul(
                            pgs[:, cg, s2 * 128:(s2 + 1) * 128],
                            lhsT=piece[:, cg * 128:(cg + 1) * 128],
                            rhs=identb[:, :],
                            start=(pi == 0), stop=(pi == len(pieces) - 1))
            gsb = gsbp.tile([128, 2, 256], BF16, tag="gsb")
            nc.scalar.copy(gsb[:], pgs[:])
            for cg in range(2):
                for og in range(2):
                    # one PSUM zero-region (2KB bank) holds both og halves:
                    # a single start arms the whole region; og=1's first
                    # touch consumes the pending-zero, so it must NOT issue
                    # its own start (that would re-arm og=0's bytes and lose
                    # its t=0 contribution).  One stop at the global end.
                    nc.tensor.matmul(
                        pouts[:, og, :],
                        lhsT=wc[:, cg, t, og, :],
                        rhs=gsb[:, cg, :],
                        start=(t == 0 and cg == 0 and og == 0),
                        stop=(t == 8 and cg == 1 and og == 1),
                        skip_group_check=True,
                    )
        off = n * 512 + h * 256
        nc.scalar.copy(
            _rap(outsb[:], off, [[2048, 2], [1, 256]]),
            pouts[:])

    stage_a(0)
    go_cur = emit_gather(0, 0)
    go_nxt = emit_gather(0, 1)
    for n in range(4):
        if n < 3:
            stage_a(n + 1)
        for h in range(2):
            go, go_cur = go_cur, None
            emit_compute(n, h, go)
            # issue the next gather as soon as this half-chunk's reads are done
            nxt = (n * 2 + h + 2)
            if nxt < 8:
                go_cur, go_nxt = go_nxt, emit_gather(nxt // 2, nxt % 2)
            else:
                go_cur, go_nxt = go_nxt, None
        nc.sync.dma_start(out=out[:, :, n * 512:(n + 1) * 512],
                          in_=outsb[:, :, n * 512:(n + 1) * 512])

    for p in (gsbp, wpool, gpool, po_pool, pg_pool, pti_pool, pt_pool,
              pom_pool, chain_pool, mp_pool, singles):
        p.release()


def _build():
    if "nc" in _BUILD_CACHE:
        return _BUILD_CACHE["nc"]
    nc = bacc.Bacc("TRN2", target_bir_lowering=False, debug=False, num_devices=8)
    with tile.TileContext(nc) as tc:
        _emit(tc)
    nc.compile()
    _BUILD_CACHE["nc"] = nc
    return nc


def _host_prep(x, w_off, b_off, w_conv):
    x = np.asarray(x, np.float32)
    w_off = np.asarray(w_off, np.float32)
    b_off = np.asarray(b_off, np.float32)
    w_conv = np.asarray(w_conv, np.float32)

    wof = w_off.reshape(27, 2, 128, 9)                       # [j, cg, cp, k]
    woff_sb = np.ascontiguousarray(
        np.transpose(wof, (2, 1, 3, 0))).astype(ml_dtypes.bfloat16)
    wcv = w_conv.reshape(2, 128, 2, 128, 9)                  # [og, op, cg, cp, k]
    wconv_sb = np.ascontiguousarray(
        np.transpose(wcv, (3, 2, 4, 0, 1))).astype(ml_dtypes.bfloat16)
    boff_sb = np.ascontiguousarray(b_off.reshape(27, 1))

    # wrapped-replicated base tables [p128, n4, sub4, ph8, t9], pre-shifted by -M0
    pl = np.arange(128)[:, None, None, None, None] % 16
    nv = np.arange(4)[None, :, None, None, None]
    sv = np.arange(4)[None, None, :, None, None]
    phv = np.arange(8)[None, None, None, :, None]
    tv = np.arange(9)[None, None, None, None, :]
    pos = (nv * 4 + sv) * 128 + phv * 16 + pl                # [128,4,4,8,1]
    kh = tv // 3
    kw = tv % 3
    selw_bf = np.ascontiguousarray(
        (np.arange(128)[None, :] % 16 == np.arange(16)[:, None])
        .astype(ml_dtypes.bfloat16))

    in_maps = []
    for b in range(B):
        xcl = np.zeros((HP, WP, 256), np.float32)
        xcl[2:66, 2:66, :] = np.transpose(x[b], (1, 2, 0))
        xcl_bf = xcl.astype(ml_dtypes.bfloat16).reshape(NROW, 256)
        # row-pair interleave: E[r] = concat(xcl[r], xcl[r+WP])
        xrp = np.zeros((NROW, 512), ml_dtypes.bfloat16)
        xrp[:, 0:256] = xcl_bf
        xrp[:NROW - WP, 256:512] = xcl_bf[WP:]
        xrp = np.ascontiguousarray(xrp)
        for half in range(2):
            h0 = half * 32
            hh = h0 + pos // 64
            ww = pos % 64
            bym = np.ascontiguousarray(
                np.broadcast_to(hh + kh + 1, (128, 4, 4, 8, 9)).astype(np.float64)
                - M0).astype(np.float32)
            bxm = np.ascontiguousarray(
                np.broadcast_to(ww + kw + 1, (128, 4, 4, 8, 9)).astype(np.float64)
                - M0).astype(np.float32)
            xslab = np.zeros((256, 34, 66), np.float32)
            r_lo = h0 - 1
            src_lo, src_hi = max(r_lo, 0), min(h0 + 33, H)
            xslab[:, src_lo - r_lo: src_hi - r_lo, 1:65] = x[b][:, src_lo:src_hi, :]
            xslab_sb = np.ascontiguousarray(
                np.transpose(xslab.reshape(2, 128, 34, 66), (1, 0, 2, 3))
                .reshape(128, 4488)
            ).astype(ml_dtypes.bfloat16)
            in_maps.append({
                "xslab": xslab_sb,
                "xrp": xrp,
                "woff": woff_sb,
                "wconv": wconv_sb,
                "boff": boff_sb,
                "bymt": bym,
                "bxmt": bxm,
                "selwt": selw_bf,
            })
    return in_maps


def kernel(**inputs):
    x = np.asarray(inputs["x"])
    in_maps = _host_prep(x, inputs["w_off"], inputs["b_off"], inputs["w_conv"])
    nc = _build()
    res = run_bass_kernel_spmd(nc, in_maps, core_ids=list(range(8)))
    out = np.zeros((B, COUT, H, W), np.float32)
    for core in range(8):
        b, half = core // 2, core % 2
        r = np.asarray(res.results[core]["out"]).astype(np.float32)
        o = np.transpose(r, (1, 0, 2)).reshape(COUT, 32, 64)
        out[b, :, half * 32:(half + 1) * 32, :] = o
    return out
